# revision 18
# baseline (speedup 1.0000x reference)
"""PosAttention Trainium2 kernel.

Reference computation (per batch element b):
    q   = pos @ W_q                     [n, h*d]
    sim = einsum(q, k_latent) * scale   [h, j, n]   softmax over n
    out = (softmax(sim) @ x) concat-heads @ W_out + b_out   [j, 64]

Device strategy (data-parallel over batch, one batch element per NeuronCore):
    Host folds W_q and k_latent into M[p, h*j] (weights-only transform) so
    sim^T[n, hj] = pos^T-chunks (stationary) @ M (moving) on the PE in bf16.
    ScalarE exponentiates PSUM -> E bf16 (max|sim| ~ 8, no max-subtraction
    needed). Numerator and denominator come from one x_aug = [x | 1]
    stationary matmul accumulating NT[65, hj-block] over n-chunks. DVE
    reciprocal + GpSimd partition-broadcast + DVE multiply normalize, and 64
    accumulating K=64 matmuls against W_out produce out^T[64, 256] per core.
"""

import os
import numpy as np
import ml_dtypes

# Problem shapes (nn_PosAttention: B=8, N=1024, pos_dim=128, h=d=64, j=256)
B = 8
N = 1024
P = 128
H = 64
D = 64
J = 256
XD = 64
HJ = H * J          # 16384
SCALE = D ** -0.5

NCORES = 8
WA = 1024           # sim psum / exp tile width (hj columns)
WB = 512            # NT accumulation block width (= 2 heads)
NCHUNKS = N // 128  # 8
NGROUPS = HJ // WA  # 16

_CACHE = {}


def _build_nc():
    import concourse.tile as tile
    from concourse import bacc, mybir

    bf16 = mybir.dt.bfloat16
    f32 = mybir.dt.float32

    nc = bacc.Bacc(None)

    posT_d = nc.declare_dram_parameter("posT", [P, N], bf16, isOutput=False)
    xaug_d = nc.declare_dram_parameter("xaug", [N, XD + 1], bf16, isOutput=False)
    m_d = nc.declare_dram_parameter("m", [P, HJ], bf16, isOutput=False)
    wout_d = nc.declare_dram_parameter("wout", [D, H * D], bf16, isOutput=False)
    outT_d = nc.declare_dram_parameter("outT", [D, J], f32, isOutput=True)

    BATCH = 2  # NT blocks per reciprocal batch (1 group)

    with tile.TileContext(nc) as tc:
        with (
            tc.tile_pool(name="consts", bufs=1) as consts,
            tc.tile_pool(name="work", bufs=3) as work,
            tc.tile_pool(name="norm", bufs=4) as norm,
            tc.tile_pool(name="ntsb", bufs=8) as ntsbp,
            tc.tile_pool(name="ps", bufs=2, space="PSUM") as ps,
            tc.tile_pool(name="psnt", bufs=3, space="PSUM") as psnt,
            tc.tile_pool(name="psout", bufs=1, space="PSUM") as psout,
        ):
    # ---- constant loads, in consumption order ----
            xaug_sb = consts.tile([128, NCHUNKS, XD + 1], bf16)
            nc.sync.dma_start(
                out=xaug_sb,
                in_=xaug_d[:, :].rearrange("(c p) d -> p c d", p=128),
            )

            posT_sb = consts.tile([P, N], bf16)
            nc.sync.dma_start(out=posT_sb, in_=posT_d[:, :])

            m_sb = [
                consts.tile([P, WA], bf16, tag=f"m{g}", name=f"m_sb{g}")
                for g in range(NGROUPS)
            ]
            nc.sync.dma_start(out=m_sb[0], in_=m_d[:, 0:WA])
            nc.sync.dma_start(out=m_sb[1], in_=m_d[:, WA:2 * WA])

            wall_sb = consts.tile([D, H * D], bf16)
            nc.sync.dma_start(out=wall_sb, in_=wout_d[:, :])

            for g in range(2, NGROUPS):
                nc.sync.dma_start(out=m_sb[g], in_=m_d[:, g * WA:(g + 1) * WA])

            outT_ps = psout.tile([D, J], f32)

            # tiny dummy exp so the ACT table load overlaps the prologue DMAs
            warm = consts.tile([1, 1], f32)
            nc.vector.memset(warm, 0.0)
            warm2 = consts.tile([1, 1], f32)
            nc.scalar.activation(
                out=warm2, in_=warm, func=mybir.ActivationFunctionType.Exp,
            )

            def flush_norm(pending, fast_tail=False):
                """Batched reciprocal + normalize + out-projection.

                DVE reciprocal costs 8 cycles per free-dim element, so 1/D on
                a [1, 512] row is ~3.3us.  Instead gather the D rows of a
                batch into [128, 4*batch] via DMA partition-redistribution,
                reciprocate there (free dim = 4*batch), and scatter back.
                """
                nb = len(pending)
                rrs = []
                if fast_tail:
                    # latency-optimized path for the kernel tail: a direct
                    # [1, 512] DVE reciprocal costs ~3.4us of DVE but skips
                    # two DMA+semaphore hops of the batched path
                    for i, (_, ntsb) in enumerate(pending):
                        rr = norm.tile([1, WB], f32, tag="rr")
                        nc.vector.reciprocal(out=rr, in_=ntsb[XD:XD + 1, :])
                        rrs.append(rr)
                else:
                    dt = norm.tile([128, 4 * BATCH], f32, tag="dt")
                    for i, (_, ntsb) in enumerate(pending):
                        # [1, 512] row -> [128, 4] partition spread (sizes
                        # match; DMA iterates each AP in its own order)
                        nc.sync.dma_start(
                            out=dt[:, 4 * i:4 * (i + 1)],
                            in_=ntsb[XD:XD + 1, :],
                        )
                    rec = norm.tile([128, 4 * BATCH], f32, tag="rec")
                    nc.vector.reciprocal(out=rec[:, :4 * nb], in_=dt[:, :4 * nb])
                    for i in range(nb):
                        rr = norm.tile([1, WB], f32, tag="rr")
                        nc.sync.dma_start(out=rr, in_=rec[:, 4 * i:4 * (i + 1)])
                        rrs.append(rr)
                for i, (head0, ntsb) in enumerate(pending):
                    rb = norm.tile([XD, WB], f32, tag="rb")
                    nc.gpsimd.partition_broadcast(rb, rrs[i])
                    ntn = norm.tile([XD, WB], bf16, tag="ntn")
                    nc.vector.tensor_mul(out=ntn, in0=ntsb[0:XD, :], in1=rb)
                    for hh in range(WB // J):
                        head = head0 + hh
                        nc.tensor.matmul(
                            out=outT_ps,
                            lhsT=wall_sb[:, head * D:(head + 1) * D],
                            rhs=ntn[:, hh * J:(hh + 1) * J],
                            start=(head == 0), stop=(head == H - 1),
                            skip_group_check=True,
                        )
                pending.clear()

            # ---- main loop over hj column groups ----
            pending = []
            for g in range(NGROUPS):
                nt_a = psnt.tile([XD + 1, WB], f32, tag="nt", name=f"nt_a{g}")
                nt_b = psnt.tile([XD + 1, WB], f32, tag="nt", name=f"nt_b{g}")
                for nch in range(NCHUNKS):
                    sim = ps.tile([128, WA], f32, tag="sim")
                    lhs = posT_sb[:, nch * 128:(nch + 1) * 128]
                    for s in range(WA // 512):
                        nc.tensor.matmul(
                            out=sim[:, s * 512:(s + 1) * 512],
                            lhsT=lhs,
                            rhs=m_sb[g][:, s * 512:(s + 1) * 512],
                            start=True, stop=True,
                        )
                    e_t = work.tile([128, WA], bf16, tag="e")
                    nc.scalar.activation(
                        out=e_t, in_=sim, func=mybir.ActivationFunctionType.Exp,
                    )
                    first, last = nch == 0, nch == NCHUNKS - 1
                    nc.tensor.matmul(
                        out=nt_a, lhsT=xaug_sb[:, nch, :], rhs=e_t[:, 0:WB],
                        start=first, stop=last, skip_group_check=True,
                    )
                    nc.tensor.matmul(
                        out=nt_b, lhsT=xaug_sb[:, nch, :], rhs=e_t[:, WB:WA],
                        start=first, stop=last, skip_group_check=True,
                    )

                # free the NT psum slots immediately with one DVE copy each
                for half, nt in ((0, nt_a), (1, nt_b)):
                    ntsb = ntsbp.tile([XD + 1, WB], f32, tag="ntsb",
                                      name=f"ntsb{g}_{half}")
                    nc.vector.tensor_copy(out=ntsb, in_=nt)
                    pending.append((g * (WA // J) + half * (WB // J), ntsb))
                if len(pending) >= BATCH:
                    flush_norm(pending, fast_tail=(g == NGROUPS - 1))
            if pending:
                flush_norm(pending, fast_tail=True)

            outT_sb = consts.tile([D, J], f32)
            nc.vector.tensor_copy(out=outT_sb, in_=outT_ps)
            nc.sync.dma_start(out=outT_d[:, :], in_=outT_sb)

    nc.finalize()
    return nc


def _install_ntff_shim():
    """The image's antenv lacks axon_hooks; recreate it so trace=True works."""
    import sys
    import types
    try:
        from antenv.axon_hooks import get_axon_ntff_profile_hook  # noqa: F401
        return
    except ImportError:
        pass
    try:
        import antenv
        from trn_agent_boot.trn_boot import _ntff_profile_via_ctypes
        mod = types.ModuleType("antenv.axon_hooks")
        state = {"h": None}
        mod.set_axon_ntff_profile_hook = lambda h: state.__setitem__("h", h)
        mod.get_axon_ntff_profile_hook = lambda: state["h"]
        sys.modules["antenv.axon_hooks"] = mod
        antenv.axon_hooks = mod
        hook = _ntff_profile_via_ctypes("/opt/axon/libaxon_pjrt.so")
        mod.set_axon_ntff_profile_hook(hook)
    except Exception as e:  # degrade to no-trace
        print(f"ntff shim failed ({e}); tracing disabled", flush=True)


def _get_nc():
    if "nc" not in _CACHE:
        _CACHE["nc"] = _build_nc()
    return _CACHE["nc"]


def kernel(pos, x, W_q, k_latent, W_out, b_out):
    from concourse.bass_utils import run_bass_kernel_spmd

    bf = ml_dtypes.bfloat16

    # Weight folding (data-independent): M[p, h*j] = sum_d W_q[p, h*64+d] k[j, d] * scale
    m = np.einsum(
        "phd,jd->phj",
        np.asarray(W_q, np.float32).reshape(P, H, D),
        np.asarray(k_latent, np.float32),
    ).reshape(P, HJ) * SCALE
    m = np.ascontiguousarray(m, dtype=np.float32).astype(bf)

    # W_out[h*64+d', o] -> [d', h*64+o]
    wall = np.ascontiguousarray(
        np.asarray(W_out, np.float32).reshape(H, D, D).transpose(1, 0, 2).reshape(D, H * D)
    ).astype(bf)

    pos = np.asarray(pos, np.float32)
    x = np.asarray(x, np.float32)
    ones = np.ones((N, 1), np.float32)

    in_maps = []
    for b in range(B):
        in_maps.append({
            "posT": np.ascontiguousarray(pos[b].T).astype(bf),
            "xaug": np.concatenate([x[b], ones], axis=1).astype(bf),
            "m": m,
            "wout": wall,
        })

    nc = _get_nc()
    trace = os.environ.get("KERNEL_TRACE") == "1"
    if trace:
        _install_ntff_shim()
    res = run_bass_kernel_spmd(nc, in_maps, core_ids=list(range(NCORES)), trace=trace)
    if trace:
        print(f"HW exec time: {res.exec_time_ns} ns", flush=True)
        _CACHE["last_result"] = res

    b_out = np.asarray(b_out, np.float32)
    out = np.stack([res.results[b]["outT"].T + b_out for b in range(B)])
    return out.astype(np.float32)


# revision 30
# speedup vs baseline: 1.0350x; 1.0350x over previous
"""PosAttention Trainium2 kernel.

Reference computation (per batch element b):
    q   = pos @ W_q                     [n, h*d]
    sim = einsum(q, k_latent) * scale   [h, j, n]   softmax over n
    out = (softmax(sim) @ x) concat-heads @ W_out + b_out   [j, 64]

Device strategy (data-parallel over batch, one batch element per NeuronCore):
    Host folds W_q and k_latent into M[p, h*j] (weights-only transform) so
    sim^T[n, hj] = pos^T-chunks (stationary) @ M (moving) on the PE in bf16.
    ScalarE exponentiates PSUM -> E bf16 (max|sim| ~ 8, no max-subtraction
    needed). Numerator and denominator come from one x_aug = [x | 1]
    stationary matmul accumulating NT[65, hj-block] over n-chunks. DVE
    reciprocal + GpSimd partition-broadcast + DVE multiply normalize, and 64
    accumulating K=64 matmuls against W_out produce out^T[64, 256] per core.
"""

import os
import numpy as np
import ml_dtypes

# Problem shapes (nn_PosAttention: B=8, N=1024, pos_dim=128, h=d=64, j=256)
B = 8
N = 1024
P = 128
H = 64
D = 64
J = 256
XD = 64
HJ = H * J          # 16384
SCALE = D ** -0.5

NCORES = 8
WA = 1024           # sim psum / exp tile width (hj columns)
WB = 512            # NT accumulation block width (= 2 heads)
NCHUNKS = N // 128  # 8
NGROUPS = HJ // WA  # 16

_CACHE = {}


def _build_nc():
    import concourse.tile as tile
    from concourse import bacc, mybir

    bf16 = mybir.dt.bfloat16
    f32 = mybir.dt.float32

    nc = bacc.Bacc(None)

    posT_d = nc.declare_dram_parameter("posT", [P, N], bf16, isOutput=False)
    xaug_d = nc.declare_dram_parameter("xaug", [N, XD + 1], bf16, isOutput=False)
    m_d = nc.declare_dram_parameter("m", [P, HJ], bf16, isOutput=False)
    wout_d = nc.declare_dram_parameter("wout", [D, H * D], bf16, isOutput=False)
    outT_d = nc.declare_dram_parameter("outT", [D, J], f32, isOutput=True)

    BATCH = 2  # NT blocks per reciprocal batch (1 group)

    with tile.TileContext(nc) as tc:
        with (
            tc.tile_pool(name="consts", bufs=1) as consts,
            tc.tile_pool(name="work", bufs=3) as work,
            tc.tile_pool(name="norm", bufs=4) as norm,
            tc.tile_pool(name="ntsb", bufs=8) as ntsbp,
            tc.tile_pool(name="ps", bufs=2, space="PSUM") as ps,
            tc.tile_pool(name="psnt", bufs=3, space="PSUM") as psnt,
            tc.tile_pool(name="psout", bufs=1, space="PSUM") as psout,
        ):
    # ---- constant loads ----
            # The Sync sequencer pays ~600ns of issue time per dma_start, so
            # only the first-needed tensors go on it; the rest issue from the
            # (otherwise idle) GpSimd queue.
            posT_sb = consts.tile([P, N], bf16)
            nc.sync.dma_start(out=posT_sb, in_=posT_d[:, :])

            m_sb = [
                consts.tile([P, WA], bf16, tag=f"m{g}", name=f"m_sb{g}")
                for g in range(NGROUPS)
            ]
            nc.sync.dma_start(out=m_sb[0], in_=m_d[:, 0:WA])
            nc.sync.dma_start(out=m_sb[1], in_=m_d[:, WA:2 * WA])

            xaug_sb = consts.tile([128, NCHUNKS, XD + 1], bf16)
            nc.gpsimd.dma_start(
                out=xaug_sb,
                in_=xaug_d[:, :].rearrange("(c p) d -> p c d", p=128),
            )

            wall_sb = consts.tile([D, H * D], bf16)
            nc.gpsimd.dma_start(out=wall_sb, in_=wout_d[:, :])

            outT_ps = psout.tile([D, J], f32)

            # tiny dummy exp so the ACT table load overlaps the prologue DMAs
            warm = consts.tile([1, 1], f32)
            nc.vector.memset(warm, 0.0)
            warm2 = consts.tile([1, 1], f32)
            nc.scalar.activation(
                out=warm2, in_=warm, func=mybir.ActivationFunctionType.Exp,
            )

            def flush_norm(pending, fast_tail=False):
                """Batched reciprocal + normalize + out-projection.

                DVE reciprocal costs 8 cycles per free-dim element, so 1/D on
                a [1, 512] row is ~3.3us.  Instead gather the D rows of a
                batch into [128, 4*batch] via DMA partition-redistribution,
                reciprocate there (free dim = 4*batch), and scatter back.
                """
                nb = len(pending)
                rrs = []
                if fast_tail:
                    # latency-optimized path for the kernel tail: a direct
                    # [1, 512] DVE reciprocal costs ~3.4us of DVE but skips
                    # two DMA+semaphore hops of the batched path
                    for i, (_, ntsb) in enumerate(pending):
                        rr = norm.tile([1, WB], f32, tag="rr")
                        nc.vector.reciprocal(out=rr, in_=ntsb[XD:XD + 1, :])
                        rrs.append(rr)
                else:
                    dt = norm.tile([128, 4 * BATCH], f32, tag="dt")
                    for i, (_, ntsb) in enumerate(pending):
                        # [1, 512] row -> [128, 4] partition spread (sizes
                        # match; DMA iterates each AP in its own order)
                        nc.sync.dma_start(
                            out=dt[:, 4 * i:4 * (i + 1)],
                            in_=ntsb[XD:XD + 1, :],
                        )
                    rec = norm.tile([128, 4 * BATCH], f32, tag="rec")
                    nc.vector.reciprocal(out=rec[:, :4 * nb], in_=dt[:, :4 * nb])
                    for i in range(nb):
                        rr = norm.tile([1, WB], f32, tag="rr")
                        nc.sync.dma_start(out=rr, in_=rec[:, 4 * i:4 * (i + 1)])
                        rrs.append(rr)
                for i, (head0, ntsb) in enumerate(pending):
                    rb = norm.tile([XD, WB], f32, tag="rb")
                    nc.gpsimd.partition_broadcast(rb, rrs[i])
                    ntn = norm.tile([XD, WB], bf16, tag="ntn")
                    nc.vector.tensor_mul(out=ntn, in0=ntsb[0:XD, :], in1=rb)
                    for hh in range(WB // J):
                        head = head0 + hh
                        nc.tensor.matmul(
                            out=outT_ps,
                            lhsT=wall_sb[:, head * D:(head + 1) * D],
                            rhs=ntn[:, hh * J:(hh + 1) * J],
                            start=(head == 0), stop=(head == H - 1),
                            skip_group_check=True,
                        )
                pending.clear()

            # ---- main loop over hj column groups ----
            pending = []
            for g in range(NGROUPS):
                if g + 2 < NGROUPS:
                    gg = g + 2
                    nc.gpsimd.dma_start(
                        out=m_sb[gg], in_=m_d[:, gg * WA:(gg + 1) * WA])
                nt_a = psnt.tile([XD + 1, WB], f32, tag="nt", name=f"nt_a{g}")
                nt_b = psnt.tile([XD + 1, WB], f32, tag="nt", name=f"nt_b{g}")
                for nch in range(NCHUNKS):
                    sim = ps.tile([128, WA], f32, tag="sim")
                    lhs = posT_sb[:, nch * 128:(nch + 1) * 128]
                    for s in range(WA // 512):
                        nc.tensor.matmul(
                            out=sim[:, s * 512:(s + 1) * 512],
                            lhsT=lhs,
                            rhs=m_sb[g][:, s * 512:(s + 1) * 512],
                            start=True, stop=True,
                        )
                    e_t = work.tile([128, WA], bf16, tag="e")
                    nc.scalar.activation(
                        out=e_t, in_=sim, func=mybir.ActivationFunctionType.Exp,
                    )
                    first, last = nch == 0, nch == NCHUNKS - 1
                    nc.tensor.matmul(
                        out=nt_a, lhsT=xaug_sb[:, nch, :], rhs=e_t[:, 0:WB],
                        start=first, stop=last, skip_group_check=True,
                    )
                    nc.tensor.matmul(
                        out=nt_b, lhsT=xaug_sb[:, nch, :], rhs=e_t[:, WB:WA],
                        start=first, stop=last, skip_group_check=True,
                    )

                # free the NT psum slots immediately with one DVE copy each
                for half, nt in ((0, nt_a), (1, nt_b)):
                    ntsb = ntsbp.tile([XD + 1, WB], f32, tag="ntsb",
                                      name=f"ntsb{g}_{half}")
                    nc.vector.tensor_copy(out=ntsb, in_=nt)
                    pending.append((g * (WA // J) + half * (WB // J), ntsb))
                if len(pending) >= BATCH:
                    flush_norm(pending)
            if pending:
                flush_norm(pending)

            outT_sb = consts.tile([D, J], f32)
            nc.vector.tensor_copy(out=outT_sb, in_=outT_ps)
            nc.sync.dma_start(out=outT_d[:, :], in_=outT_sb)

    nc.finalize()
    return nc


def _install_ntff_shim():
    """The image's antenv lacks axon_hooks; recreate it so trace=True works."""
    import sys
    import types
    try:
        from antenv.axon_hooks import get_axon_ntff_profile_hook  # noqa: F401
        return
    except ImportError:
        pass
    try:
        import antenv
        from trn_agent_boot.trn_boot import _ntff_profile_via_ctypes
        mod = types.ModuleType("antenv.axon_hooks")
        state = {"h": None}
        mod.set_axon_ntff_profile_hook = lambda h: state.__setitem__("h", h)
        mod.get_axon_ntff_profile_hook = lambda: state["h"]
        sys.modules["antenv.axon_hooks"] = mod
        antenv.axon_hooks = mod
        hook = _ntff_profile_via_ctypes("/opt/axon/libaxon_pjrt.so")
        mod.set_axon_ntff_profile_hook(hook)
    except Exception as e:  # degrade to no-trace
        print(f"ntff shim failed ({e}); tracing disabled", flush=True)


def _get_nc():
    if "nc" not in _CACHE:
        _CACHE["nc"] = _build_nc()
    return _CACHE["nc"]


def kernel(pos, x, W_q, k_latent, W_out, b_out):
    from concourse.bass_utils import run_bass_kernel_spmd

    bf = ml_dtypes.bfloat16

    # Weight folding (data-independent): M[p, h*j] = sum_d W_q[p, h*64+d] k[j, d] * scale
    m = np.einsum(
        "phd,jd->phj",
        np.asarray(W_q, np.float32).reshape(P, H, D),
        np.asarray(k_latent, np.float32),
    ).reshape(P, HJ) * SCALE
    m = np.ascontiguousarray(m, dtype=np.float32).astype(bf)

    # W_out[h*64+d', o] -> [d', h*64+o]
    wall = np.ascontiguousarray(
        np.asarray(W_out, np.float32).reshape(H, D, D).transpose(1, 0, 2).reshape(D, H * D)
    ).astype(bf)

    pos = np.asarray(pos, np.float32)
    x = np.asarray(x, np.float32)
    ones = np.ones((N, 1), np.float32)

    in_maps = []
    for b in range(B):
        in_maps.append({
            "posT": np.ascontiguousarray(pos[b].T).astype(bf),
            "xaug": np.concatenate([x[b], ones], axis=1).astype(bf),
            "m": m,
            "wout": wall,
        })

    nc = _get_nc()
    trace = os.environ.get("KERNEL_TRACE") == "1"
    if trace:
        _install_ntff_shim()
    res = run_bass_kernel_spmd(nc, in_maps, core_ids=list(range(NCORES)), trace=trace)
    if trace:
        print(f"HW exec time: {res.exec_time_ns} ns", flush=True)
        _CACHE["last_result"] = res

    b_out = np.asarray(b_out, np.float32)
    out = np.stack([res.results[b]["outT"].T + b_out for b in range(B)])
    return out.astype(np.float32)
